# revision 8
# baseline (speedup 1.0000x reference)
"""Trainium2 Bass kernel for AnsiToPixels (embedding_lookup, memory-bound).

Computation (per glyph cell):
  raw[y,x]  = sum_ch char[ch] * glyph[ch,y,x]          (256-ch dense "one-hot" matmul)
  fg[c]     = (0.5*fg_bold+0.5) * fg_color[c]
  bg[c]     = (0.5*bg_bold+0.5) * bg_color[c]
  out[y,x,c] = raw[y,x]*(fg[c]-bg[c]) + bg[c]

Sharding: pure data parallelism over batch B=128 -> 16 per core on 8 cores,
glyph table replicated. Each core processes 25600 cells.

Layout: cells are assigned to partitions with an OCT=20-way interleave.
A macro-tile covers 2560 consecutive cells; partition p owns the 20
consecutive cells {20p+j}. Benefits:
  - char DMA-in: one DMA per macro-tile, 20*1056B = 21KB contiguous per
    partition (full line rate).
  - d/bg color math: batched ops slicing the big char tile directly.
  - out DMA: partition p's per-y output span is 20*8*3*4 = 1920B contiguous
    in DRAM (20 divides the 80-cell row). Rows advance every 4 partitions;
    4 DMAs per macro-tile (partition stride 4) each span 32 partitions ->
    all 16 SDMA engines, 3-dim DRAM APs.

Per macro-tile j-substream (128 cells): 2x PE transpose-mode [128,128]
(channel-major charT in PSUM), ACT copy+cast to SBUF bf16, 2x bf16 matmul
-> raw[cell,pix] in PSUM f32, then a fused per-channel blend
out = raw*d + bg via tensor_scalar (DVE) / activation-Identity (ACT) with
per-partition scalars.
"""

import os
import sys

import numpy as np

for _p in ("/opt/trn_rl_repo", "/root/.axon_site/_ro/trn_rl_repo"):
    if os.path.isdir(_p) and _p not in sys.path:
        sys.path.insert(0, _p)

import concourse.bass as bass  # noqa: E402
import concourse.mybir as mybir  # noqa: E402
import concourse.tile as tile  # noqa: E402
from concourse import bacc  # noqa: E402
from concourse.bass_utils import run_bass_kernel_spmd  # noqa: E402
from concourse.masks import make_identity  # noqa: E402


def _ensure_ntff_hook():
    """Register the axon NTFF profile hook if the image's antenv lacks it,
    so run_bass_kernel_spmd(trace=True) can capture HW exec time."""
    try:
        from antenv.axon_hooks import get_axon_ntff_profile_hook  # noqa: F401

        return
    except ImportError:
        pass
    try:
        import types

        import antenv
        from trn_agent_boot.trn_boot import _ntff_profile_via_ctypes

        hook = _ntff_profile_via_ctypes("/opt/axon/libaxon_pjrt.so")
        mod = types.ModuleType("antenv.axon_hooks")
        mod.get_axon_ntff_profile_hook = lambda: hook
        mod.set_axon_ntff_profile_hook = lambda h: None
        sys.modules["antenv.axon_hooks"] = mod
        antenv.axon_hooks = mod
    except Exception as e:  # profiling is best-effort
        print(f"NTFF hook registration failed: {e}", file=sys.stderr)


N_CORES = 8
B = 128
GRID_H, GRID_W = 20, 80
GLYPH_H, GLYPH_W = 16, 8
N_GLYPHS = 256
PIX = GLYPH_H * GLYPH_W  # 128

B_SHARD = B // N_CORES  # 16
CELLS = B_SHARD * GRID_H * GRID_W  # 25600
OCT = 20  # cells per partition (consecutive within a row)
MT = 128 * OCT  # cells per macro-tile (2560)
NT = CELLS // MT  # 10 macro-tiles
OPR = GRID_W // OCT  # cell-groups per image row (4)
ROWS = B_SHARD * GRID_H  # 320 image (b,h) rows per core
RPT = 128 // OPR  # image rows per macro-tile (32)

F32 = mybir.dt.float32
BF16 = mybir.dt.bfloat16


def _bcast_last(ap, n):
    """Append a stride-0 dim of size n to an AP (free-dim broadcast)."""
    return bass.AP(tensor=ap.tensor, offset=ap.offset, ap=[*ap.ap, [0, n]])


def build_kernel():
    nc = bacc.Bacc(
        "TRN2",
        target_bir_lowering=False,
        debug=False,
        enable_asserts=False,
        num_devices=N_CORES,
    )
    data = nc.dram_tensor("data", [CELLS, 264], F32, kind="ExternalInput").ap()
    glyph = nc.dram_tensor("glyph", [N_GLYPHS, PIX], F32, kind="ExternalInput").ap()
    outp = nc.dram_tensor(
        "out", [ROWS, GLYPH_H, GRID_W, GLYPH_W * 3], BF16, kind="ExternalOutput"
    ).ap()
    # data viewed as [tile, p, j, ch]: cell = t*MT + p*OCT + j
    data_t = data.rearrange("(t p j) ch -> t p j ch", p=128, j=OCT)

    with tile.TileContext(nc) as tc:
        with (
            tc.tile_pool(name="const", bufs=1) as const,
            tc.tile_pool(name="char", bufs=2) as char_pool,
            tc.tile_pool(name="ctbf", bufs=4) as ctbf_pool,
            tc.tile_pool(name="outsb", bufs=2) as out_pool,
            tc.tile_pool(name="grp", bufs=2) as grp_pool,
            tc.tile_pool(name="psT", bufs=3, space="PSUM") as psT,
            tc.tile_pool(name="psR", bufs=5, space="PSUM") as psR,
        ):
            ident = const.tile([128, 128], F32)
            make_identity(nc, ident[:, :])

            g32 = const.tile([128, 256], F32)
            nc.sync.dma_start(out=g32[:, 0:128], in_=glyph[0:128, :])
            nc.sync.dma_start(out=g32[:, 128:256], in_=glyph[128:256, :])
            gbf = const.tile([128, 256], BF16)
            nc.scalar.copy(gbf[:, :], g32[:, :])

            for t in range(NT):
                # one contiguous 21KB/partition load of all 20 substreams
                char = char_pool.tile([128, OCT, 264], F32)
                nc.sync.dma_start(out=char[:, :, :], in_=data_t[t, :, :, :])

                # batched d/bg from color channels (fgb, fgc*3, bgb, bgc*3)
                sf = grp_pool.tile([128, OCT], F32, tag="sf")
                sb = grp_pool.tile([128, OCT], F32, tag="sb")
                fg = grp_pool.tile([128, OCT, 3], F32, tag="fg")
                bg = grp_pool.tile([128, OCT, 3], F32, tag="bg")
                d = grp_pool.tile([128, OCT, 3], F32, tag="d")
                nc.vector.tensor_scalar(
                    out=sf[:, :],
                    in0=char[:, :, 256],
                    scalar1=0.5,
                    scalar2=0.5,
                    op0=mybir.AluOpType.mult,
                    op1=mybir.AluOpType.add,
                )
                nc.vector.tensor_scalar(
                    out=sb[:, :],
                    in0=char[:, :, 260],
                    scalar1=0.5,
                    scalar2=0.5,
                    op0=mybir.AluOpType.mult,
                    op1=mybir.AluOpType.add,
                )
                nc.vector.tensor_mul(
                    fg[:, :, :], char[:, :, 257:260], _bcast_last(sf[:, :], 3)
                )
                nc.vector.tensor_mul(
                    bg[:, :, :], char[:, :, 261:264], _bcast_last(sb[:, :], 3)
                )
                nc.vector.tensor_sub(d[:, :, :], fg[:, :, :], bg[:, :, :])

                out_sb = out_pool.tile([128, GLYPH_H, OCT, GLYPH_W, 3], BF16)
                for j in range(OCT):
                    ctps = psT.tile([128, 256], F32)
                    nc.tensor.transpose(
                        ctps[:, 0:128], char[:, j, 0:128], ident[:, :]
                    )
                    nc.tensor.transpose(
                        ctps[:, 128:256], char[:, j, 128:256], ident[:, :]
                    )
                    ctbf = ctbf_pool.tile([128, 256], BF16)
                    nc.scalar.copy(ctbf[:, :], ctps[:, :])

                    raw = psR.tile([128, PIX], F32)
                    nc.tensor.matmul(
                        raw[:, :],
                        ctbf[:, 0:128],
                        gbf[:, 0:128],
                        start=True,
                        stop=False,
                    )
                    nc.tensor.matmul(
                        raw[:, :],
                        ctbf[:, 128:256],
                        gbf[:, 128:256],
                        start=False,
                        stop=True,
                    )

                    # blend: out_sb[p,y,j,x,c] = raw[p,(y,x)] * d[p,j,c] + bg[p,j,c]
                    rawv = raw[:, :].rearrange("p (y x) -> p y x", x=GLYPH_W)
                    nc.scalar.activation(
                        out_sb[:, :, j, :, 0],
                        rawv,
                        mybir.ActivationFunctionType.Identity,
                        bias=bg[:, j, 0:1],
                        scale=d[:, j, 0:1],
                    )
                    for c in (1, 2):
                        nc.vector.tensor_scalar(
                            out=out_sb[:, :, j, :, c],
                            in0=rawv,
                            scalar1=d[:, j, c : c + 1],
                            scalar2=bg[:, j, c : c + 1],
                            op0=mybir.AluOpType.mult,
                            op1=mybir.AluOpType.add,
                        )

                # out: 4 DMAs per macro-tile; partition stride 4 spans all
                # 16 SDMA engines, DRAM AP is 3-dim after merging
                o_view = outp[t * RPT : (t + 1) * RPT, :, :, :].rearrange(
                    "rg y (o j) k -> rg o y j k", j=OCT
                )
                for o in range(OPR):
                    nc.sync.dma_start(
                        out=o_view[:, o, :, :, :],
                        in_=out_sb[o :: OPR, :, :, :, :],
                    )

    nc.compile()
    return nc


_NC = None


def _get_nc():
    global _NC
    if _NC is None:
        _NC = build_kernel()
    return _NC


def run(data, char_matrix, trace=False):
    data = np.ascontiguousarray(np.asarray(data, dtype=np.float32))
    glyph = np.ascontiguousarray(
        np.asarray(char_matrix, dtype=np.float32).reshape(N_GLYPHS, PIX)
    )
    assert data.shape == (B, GRID_H, GRID_W, 264), data.shape

    in_maps = []
    for i in range(N_CORES):
        shard = data[i * B_SHARD : (i + 1) * B_SHARD].reshape(CELLS, 264)
        in_maps.append({"data": np.ascontiguousarray(shard), "glyph": glyph})

    nc = _get_nc()
    if trace:
        _ensure_ntff_hook()
    res = run_bass_kernel_spmd(
        nc, in_maps, core_ids=list(range(N_CORES)), trace=trace
    )
    out = np.concatenate(
        [
            np.asarray(r["out"], dtype=np.float32).reshape(
                B_SHARD, GRID_H * GLYPH_H, GRID_W * GLYPH_W, 3
            )
            for r in res.results
        ],
        axis=0,
    )
    return out, res.exec_time_ns


def kernel(data, char_matrix):
    out, _ = run(data, char_matrix, trace=False)
    return out
